# revision 19
# baseline (speedup 1.0000x reference)
"""Trainium2 Bass kernel for nn_AttentionLoss (guided attention loss).

loss = sum_{b, t<ml_b, n<tl_b} pred[b,t,n] * (1 - exp(-12.5*(n/tl_b - t/ml_b)^2))
       / sum_b (tl_b*ml_b)

Key identity: with d = n/tl - t/ml in (-1,1), the Gaussian factors into a
short Fourier cosine series,
    exp(-12.5 d^2) ~= a0 + sum_{k=1..K} a_k cos(pi k d)
                    = a0 + sum_k a_k [cos(pi k x)cos(pi k y) + sin(pi k x)sin(pi k y)]
(x = n/tl, y = t/ml; coefficients fit by least squares, K=6 gives ~3e-5
max error).  This makes the whole loss a contraction of pred over t with
R1 = 1+2K smooth per-t factor columns, i.e. pure TensorE work:

  S[r, n] = sum_t W_r(t) pred[t, n],   W = [mask_t, mask_t*cos(pi k y_t),
                                            mask_t*sin(pi k y_t)]
  loss_b  = sum_{n<tl} (1-a0) S[0,n] - sum_k a_k (cos(pi k x_n) S[k,n]
                                                  + sin(pi k x_n) S[K+k,n])

Device strategy (8 NeuronCores, data-parallel over batch):
  - Batches sorted by mel_length descending, dealt into 8 "slots" of 8
    batches; core c takes the (8s+c)-th ranked batch for slot s.  Per slot
    the program uses C_s 256-col sub-rows per partition (C_s*128 >= max ml
    in the slot): mel rows t >= ml are never transferred (about 2.2x
    traffic saving vs. the full 2000 rows).
  - pred is sent as fp8 e4m3 (4x less DMA than f32) in a host-permuted
    [128, sum(C_s), 256] layout, loaded by a few big chunked DMAs (the
    HWDGE descriptor-generation overhead is ~625ns per DMA instruction,
    so few large DMAs win; per-partition contiguity is 2KB -> full rate).
  - Matmuls use fp8 DoubleRow perf mode: each instruction contracts two
    128-partition row-groups of [128, 2, tlpad] pred against [128, 2, 16]
    factor weights, accumulating S in PSUM ([16, tlpad] f32).
  - PSUM -> SBUF copies on DVE into one [16, 8*256] tile, single DMA out;
    host applies the n-side cos/sin factors and normalizes by sum(tl*ml).
"""
import sys

sys.path.insert(0, "/opt/trn_rl_repo")

import numpy as np
import ml_dtypes

import concourse.bass as bass
import concourse.tile as tile
from concourse import bacc, mybir
from concourse.bass_utils import run_bass_kernel_spmd

B, MEL_MAX, TEXT_MAX = 64, 2000, 256
C12 = 12.5
ATTN_WEIGHT = 1.0

N_CORES = 8
SLOTS = 8                     # batch slots per core
KF = 6                        # Fourier cosine terms
R1 = 1 + 2 * KF               # weight columns: mask, cos*K, sin*K
R1P = 16                      # padded for alignment
FP8 = ml_dtypes.float8_e4m3

_COMPILED = {}


def _fourier_coefs():
    """Least-squares fit of exp(-C12 d^2) ~ a0 + sum a_k cos(pi k d) on [-1,1]."""
    d = np.linspace(-1.0, 1.0, 8001)
    g = np.exp(-C12 * d * d)
    M = np.stack([np.cos(np.pi * k * d) for k in range(KF + 1)], axis=1)
    a, *_ = np.linalg.lstsq(M, g, rcond=None)
    return a  # [KF+1]


_ACOEF = _fourier_coefs()


def _plan(text_lengths, mel_lengths):
    """Slot assignment + per-slot geometry.

    Returns (grid, cfg): grid[s][c] = batch index; cfg = tuple of
    (C_s, tlpad_s) per slot (the compile key).
    """
    tl = np.asarray(text_lengths).astype(np.int64)
    ml = np.asarray(mel_lengths).astype(np.int64)
    order = np.argsort(-ml, kind="stable")
    grid = [[int(order[8 * s + c]) for c in range(N_CORES)]
            for s in range(SLOTS)]
    cfg = []
    for s in range(SLOTS):
        bs = grid[s]
        mlmax = int(max(ml[b] for b in bs))
        tlmax = int(max(tl[b] for b in bs))
        C = max(2, -(-mlmax // 128))    # odd C -> one trailing non-DR matmul
        P = -(-mlmax // C)              # partitions actually holding t < mlmax
        tlpad = min(TEXT_MAX, tlmax + (tlmax & 1))
        cfg.append((C, P, tlpad))
    return grid, tuple(cfg)


def _build_program(cfg):
    nc = bacc.Bacc("TRN2", target_bir_lowering=False, debug=False,
                   num_devices=N_CORES)
    f32 = mybir.dt.float32
    f8 = mybir.dt.float8e4

    totc = sum(C for C, _, _ in cfg)

    pred_d = nc.dram_tensor("p", [128, totc, TEXT_MAX], f8,
                            kind="ExternalInput").ap()
    w_d = nc.dram_tensor("w", [128, totc, R1P], f8, kind="ExternalInput").ap()
    out_d = nc.dram_tensor("o", [R1P, SLOTS * TEXT_MAX], f32,
                           kind="ExternalOutput").ap()

    dr = mybir.MatmulPerfMode.DoubleRow

    with tile.TileContext(nc) as tc:
        with (
            tc.tile_pool(name="wp", bufs=1) as wp,
            tc.tile_pool(name="xp", bufs=1) as xp,
            tc.tile_pool(name="ps", bufs=4, space=bass.MemorySpace.PSUM) as ps,
            tc.tile_pool(name="op", bufs=1) as op,
        ):
            ot = op.tile([R1P, SLOTS * TEXT_MAX], f32)
            nc.any.memset(ot[:], 0)

            # one partition-clipped DMA per slot, biggest first; the last is
            # the smallest slot so the post-arrival tail (matmul+copy+out
            # DMA) is minimal.  HWDGE (625ns/instr) stays ahead of the DMA
            # device because early transfers are large.  The weights DMA goes
            # second: off the stream head, but well before matmuls need it.
            x_t = xp.tile([128, totc, TEXT_MAX], f8)
            w_t = wp.tile([128, totc, R1P], f8)
            pmax = max(P for _, P, _ in cfg)
            off = 0
            for s, (C, P, _) in enumerate(cfg):
                nc.sync.dma_start(x_t[0:P, off:off + C, :],
                                  pred_d[0:P, off:off + C, :])
                if s == 0:
                    nc.sync.dma_start(w_t[0:pmax, :, :], w_d[0:pmax, :, :])
                off += C

            off = 0
            for s, (C, P, tlpad) in enumerate(cfg):
                acc = ps.tile([R1P, TEXT_MAX], f32, name=f"acc{s}", tag="acc")
                nmm = (C + 1) // 2
                for l in range(nmm):
                    if 2 * l + 2 <= C:
                        nc.tensor.matmul(
                            acc[:, 0:tlpad],
                            w_t[0:P, off + 2 * l:off + 2 * l + 2, :],
                            x_t[0:P, off + 2 * l:off + 2 * l + 2, 0:tlpad],
                            start=(l == 0),
                            stop=(l == nmm - 1),
                            perf_mode=dr)
                    else:  # odd C: final single-row fp8 matmul
                        nc.tensor.matmul(
                            acc[:, 0:tlpad],
                            w_t[0:P, off + 2 * l, :],
                            x_t[0:P, off + 2 * l, 0:tlpad],
                            start=(l == 0),
                            stop=True)

                # alternate PSUM->SBUF copies between DVE and ACT so the
                # last slots' copies don't serialize on one engine
                if s % 2 == 1:
                    nc.vector.tensor_copy(
                        ot[:, s * TEXT_MAX:s * TEXT_MAX + tlpad],
                        acc[:, 0:tlpad])
                else:
                    nc.scalar.activation(
                        ot[:, s * TEXT_MAX:s * TEXT_MAX + tlpad],
                        acc[:, 0:tlpad],
                        mybir.ActivationFunctionType.Copy)
                off += C
                if s == SLOTS - 4:
                    # ship slots 0..4 as soon as their copies land: its HWDGE
                    # turn finishes before the tail DMA below needs it
                    nc.sync.dma_start(
                        out_d[:, 0:(SLOTS - 3) * TEXT_MAX],
                        ot[:, 0:(SLOTS - 3) * TEXT_MAX])

            # tail: the last three slots' columns
            nc.sync.dma_start(
                out_d[:, (SLOTS - 3) * TEXT_MAX:],
                ot[:, (SLOTS - 3) * TEXT_MAX:])

    nc.compile()
    return nc


def _get_program(cfg):
    if cfg not in _COMPILED:
        _COMPILED[cfg] = _build_program(cfg)
    return _COMPILED[cfg]


def _host_prep(predictions, text_lengths, mel_lengths):
    """Per-core input maps (grid/cfg recomputed deterministically)."""
    grid, cfg = _plan(text_lengths, mel_lengths)
    ml = np.asarray(mel_lengths).astype(np.int64)
    pred = np.asarray(predictions)
    totc = sum(C for C, _, _ in cfg)

    in_maps = []
    for c in range(N_CORES):
        p8 = np.zeros((128, totc, TEXT_MAX), dtype=FP8)
        w = np.zeros((128, totc, R1P), dtype=np.float32)
        off = 0
        for s, (C, _, _) in enumerate(cfg):
            b = grid[s][c]
            rows = 128 * C
            pb = pred[b]
            nkeep = min(rows, pb.shape[0])
            slab = np.zeros((rows, TEXT_MAX), dtype=FP8)
            slab[:nkeep] = pb[:nkeep].astype(FP8)
            p8[:, off:off + C, :] = slab.reshape(128, C, TEXT_MAX)

            t = (np.arange(128, dtype=np.float64)[:, None] * C
                 + np.arange(C, dtype=np.float64)[None, :])      # [128, C]
            mask = (t < ml[b]).astype(np.float64)
            y = np.pi * t / ml[b]
            w[:, off:off + C, 0] = mask
            for k in range(1, KF + 1):
                w[:, off:off + C, k] = mask * np.cos(k * y)
                w[:, off:off + C, KF + k] = mask * np.sin(k * y)
            off += C
        in_maps.append({"p": p8, "w": w.astype(FP8)})
    return in_maps


def _host_finish(outs, text_lengths, mel_lengths):
    grid, cfg = _plan(text_lengths, mel_lengths)
    tl = np.asarray(text_lengths).astype(np.int64)
    a = _ACOEF
    total = 0.0
    n_all = np.arange(TEXT_MAX, dtype=np.float64)
    for s, (C, _, tlpad) in enumerate(cfg):
        for c in range(N_CORES):
            b = grid[s][c]
            tlb = int(tl[b])
            S = np.asarray(outs[c][:, s * TEXT_MAX:(s + 1) * TEXT_MAX],
                           dtype=np.float64)  # [R1P, TEXT_MAX]
            x = np.pi * n_all[:tlb] / tl[b]
            contrib = (1.0 - a[0]) * np.sum(S[0, :tlb])
            for k in range(1, KF + 1):
                contrib -= a[k] * (np.sum(np.cos(k * x) * S[k, :tlb])
                                   + np.sum(np.sin(k * x) * S[KF + k, :tlb]))
            total += contrib
    active = float(np.sum((np.asarray(text_lengths).astype(np.int64)
                           * np.asarray(mel_lengths).astype(np.int64))
                          .astype(np.float32)))
    return np.float32(total / active * ATTN_WEIGHT)


def kernel(targets=None, predictions=None, text_lengths=None,
           mel_lengths=None, **_ignored):
    _, cfg = _plan(text_lengths, mel_lengths)
    nc = _get_program(cfg)
    in_maps = _host_prep(predictions, text_lengths, mel_lengths)
    res = run_bass_kernel_spmd(nc, in_maps, core_ids=list(range(N_CORES)))
    outs = [res.results[c]["o"] for c in range(N_CORES)]
    return _host_finish(outs, text_lengths, mel_lengths)


if __name__ == "__main__":
    rng = np.random.default_rng(0)
    preds = rng.random((B, MEL_MAX, TEXT_MAX), dtype=np.float32)
    tls = rng.integers(1, TEXT_MAX + 1, size=(B,)).astype(np.int32)
    mls = rng.integers(1, MEL_MAX + 1, size=(B,)).astype(np.int32)
    tgts = np.zeros_like(preds)
    out = kernel(targets=tgts, predictions=preds, text_lengths=tls,
                 mel_lengths=mls)
    print("kernel out:", out)
